# revision 20
# baseline (speedup 1.0000x reference)
"""Trainium2 Bass kernel for nn_BeliefStep: batched EKF predict step.

Pure data parallel over 8 NeuronCores: each core processes B/8 examples.
Per-core layout: batch tiled as [128 partitions, F examples] with each
example's components interleaved in the free dimension, so every DMA is a
single fully-contiguous HBM transfer (this kernel is HBM-bandwidth bound:
116 B/example of traffic vs ~60 flops/example).

Math per example (reference):
  ang_ = wrap(ang - th1*a1*DT) to [-pi, pi)
  v    = th0*a0*DT ; c = cos(ang_), s = sin(ang_)
  px_  = clip(px + v*c, -1, 1); py_ = clip(py + v*s, -1, 1)
  A    = I + u*E02 + w*E12   with u = -v*s, w = v*c
  P_   = sym(A P A^T + diag(q)) + 1e-6 I,  q = [e^{2*th2}, e^{2*th2}, e^{-8}]

The covariance update is algebraically pre-symmetrized: with
D01=P01+P10, D02=P02+P20, D12=P12+P21, g=u*P22, h=w*P22:
  S02 = D02/2 + g            S12 = D12/2 + h
  S00 = P00 + u*(D02+g) + q' S11 = P11 + w*(D12+h) + q'
  S01 = (D01 + u*D12 + w*D02)/2 + u*h
  S22 = P22 + q2'
which needs 9 multiplies + 12 adds per example instead of the 20+ of the
naive two-stage product, and each off-diagonal is computed once and
mirrored by the Scalar engine.

Emission is software-pipelined per tile (load / angle-head / covariance
of the previous tile / sin-consumers) so the vector engine never idles
waiting for the scalar engine's Sin results.
"""

import math
from contextlib import ExitStack

import numpy as np

B_TOTAL = 2097152
N_CORES = 8
BS = B_TOTAL // N_CORES  # 262144 per core
F = 256                  # examples per partition per tile
PART = 128
BUFS = (3, 3, 2)

DT = 0.1
PI = float(np.float32(np.pi))
TWO_PI = float(np.float32(2.0 * np.pi))
HALF_PI = float(np.float32(np.pi / 2.0))
# (q2 + 1e-6) prefolded in fp32: q2 = exp(2*-4) rounded to fp32
C22 = float(np.float32(np.float32(math.exp(-8.0)) + np.float32(1e-6)))

_CACHE = {}


def _build(bs, fcfg=F, reps=1, nocompute=False, hw_loop=0, bufs=BUFS,
           pipeline=True):
    import concourse.tile as tile
    from concourse import bacc, mybir

    F32 = mybir.dt.float32
    AL = mybir.AluOpType
    AF = mybir.ActivationFunctionType

    P = PART
    funits = bs // P
    if isinstance(fcfg, int):
        assert funits % fcfg == 0
        tiles = [(o, fcfg) for o in range(0, funits, fcfg)]
    else:
        tiles = []
        o = 0
        for fi in fcfg:
            tiles.append((o, fi))
            o += fi
        assert o == funits, (o, funits)

    nc = bacc.Bacc("TRN2", target_bir_lowering=False, debug=False)

    # bias constants for non-Copy activations must live in the const-AP pool
    for cval in (HALF_PI,):
        cten = nc.alloc_sbuf_tensor(f"const-f32-{cval}", [128, 1], F32)
        nc.gpsimd.memset(cten.ap(), cval)
        nc.const_aps.aps[(F32, cval)] = cten.ap()
    nc.all_engine_barrier()

    x_d = nc.dram_tensor("x", [bs * 3], F32, kind="ExternalInput").ap()
    P_d = nc.dram_tensor("P", [bs * 9], F32, kind="ExternalInput").ap()
    a_d = nc.dram_tensor("a", [bs * 2], F32, kind="ExternalInput").ap()
    th_d = nc.dram_tensor("theta", [bs * 3], F32, kind="ExternalInput").ap()
    xo_d = nc.dram_tensor("x_out", [bs * 3], F32, kind="ExternalOutput").ap()
    Po_d = nc.dram_tensor("P_out", [bs * 9], F32, kind="ExternalOutput").ap()

    def dview(d_ap, off, fi, dd):
        return d_ap[P * off * dd : P * (off + fi) * dd].rearrange(
            "(p m) -> p m", p=P
        )

    V = nc.vector
    S = nc.scalar

    with tile.TileContext(nc) as tc, ExitStack() as ctx:
        inp = ctx.enter_context(tc.tile_pool(name="inp", bufs=bufs[0]))
        outp = ctx.enter_context(tc.tile_pool(name="outp", bufs=bufs[1]))
        tmp = ctx.enter_context(tc.tile_pool(name="tmp", bufs=bufs[2]))

        if hw_loop > 0:
            ctx.enter_context(tc.For_i(0, hw_loop, 1))

        def emit_load(off, f):
            t = {}
            th = inp.tile([P, 3 * f], F32, tag="th")
            nc.sync.dma_start(th[:], dview(th_d, off, f, 3))
            at = inp.tile([P, 2 * f], F32, tag="at")
            nc.sync.dma_start(at[:], dview(a_d, off, f, 2))
            xt = inp.tile([P, 3 * f], F32, tag="xt")
            nc.sync.dma_start(xt[:], dview(x_d, off, f, 3))
            Pt = inp.tile([P, 9 * f], F32, tag="Pt")
            nc.sync.dma_start(Pt[:], dview(P_d, off, f, 9))
            xo = outp.tile([P, 3 * f], F32, tag="xo")
            Po = outp.tile([P, 9 * f], F32, tag="Po")
            t["off"], t["f"] = off, f
            t["x3"] = xt[:].rearrange("p (f d) -> p f d", d=3)
            t["a2"] = at[:].rearrange("p (f d) -> p f d", d=2)
            t["th3"] = th[:].rearrange("p (f d) -> p f d", d=3)
            t["P9"] = Pt[:].rearrange("p (f d) -> p f d", d=9)
            t["xo"], t["Po"] = xo, Po
            t["xo3"] = xo[:].rearrange("p (f d) -> p f d", d=3)
            t["Po9"] = Po[:].rearrange("p (f d) -> p f d", d=9)
            return t

        def emit_A1(t):
            # angle head (DVE) + transcendentals (ACT)
            f = t["f"]
            x3, a2, th3, xo3 = t["x3"], t["a2"], t["th3"], t["xo3"]
            ang = x3[:, :, 2]
            # mr = [r, m] = [th0*a0, th1*a1]
            mr = tmp.tile([P, 2 * f], F32, tag="mr")
            mr2 = mr[:].rearrange("p (f d) -> p f d", d=2)
            V.tensor_tensor(mr2, th3[:, :, 0:2], a2, AL.mult)
            # t = ang - DT*m
            t_ = tmp.tile([P, f], F32, tag="t_")
            V.scalar_tensor_tensor(t_[:], mr2[:, :, 1], -DT, ang, AL.mult, AL.add)
            # wrap to [-pi, pi): ang_ = t - 2pi*fl, fl = 1[(t+pi)>=2pi] - 1[(t+pi)<0]
            # (mod/divide are not valid V3 ops; |t| < ~8.3 so fl in {-1,0,1})
            ge1 = tmp.tile([P, f], F32, tag="ge1")
            V.tensor_scalar(ge1[:], t_[:], PI, TWO_PI, AL.add, AL.is_ge)
            lt0 = tmp.tile([P, f], F32, tag="lt0")
            V.tensor_scalar(lt0[:], t_[:], PI, 0.0, AL.add, AL.is_lt)
            ang1 = tmp.tile([P, f], F32, tag="ang1")
            V.scalar_tensor_tensor(ang1[:], ge1[:], -TWO_PI, t_[:], AL.mult, AL.add)
            V.scalar_tensor_tensor(
                xo3[:, :, 2], lt0[:], TWO_PI, ang1[:], AL.mult, AL.add
            )
            # sc = [s_neg, c] = [sin(-ang_), sin(pi/2 - |ang_|)]
            # (ACT Sin is only valid on [-pi, pi]; cos goes through |ang_|)
            sc = tmp.tile([P, 2 * f], F32, tag="sc")
            sc2 = sc[:].rearrange("p (f d) -> p f d", d=2)
            S.activation(sc2[:, :, 0], xo3[:, :, 2], AF.Sin, bias=0.0, scale=-1.0)
            babs = tmp.tile([P, f], F32, tag="babs")
            S.activation(babs[:], xo3[:, :, 2], AF.Abs)
            S.activation(sc2[:, :, 1], babs[:], AF.Sin, bias=HALF_PI, scale=-1.0)
            # v = DT * r ; q = exp(2*th2)
            rD = tmp.tile([P, f], F32, tag="rD")
            S.activation(rD[:], mr2[:, :, 0], AF.Copy, bias=0.0, scale=DT)
            q = tmp.tile([P, f], F32, tag="q")
            S.activation(q[:], th3[:, :, 2], AF.Exp, bias=0.0, scale=2.0)
            t["sc2"], t["rD"], t["q"] = sc2, rD, q

        def emit_A2(t):
            # sin-consumers (DVE)
            f = t["f"]
            x3, xo3, sc2, rD = t["x3"], t["xo3"], t["sc2"], t["rD"]
            px, py = x3[:, :, 0], x3[:, :, 1]
            # uw = [u, w] = [v*s_neg, v*c] = [-v*s, v*c]
            uw = tmp.tile([P, 2 * f], F32, tag="uw")
            uw2 = uw[:].rearrange("p (f d) -> p f d", d=2)
            V.tensor_tensor(
                uw2, rD[:].unsqueeze(-1).broadcast_to([P, f, 2]), sc2, AL.mult
            )
            # pp = [px + w, py - u]; clip both into xo lanes 0,1
            pp = tmp.tile([P, 2 * f], F32, tag="pp")
            pp2 = pp[:].rearrange("p (f d) -> p f d", d=2)
            V.tensor_tensor(pp2[:, :, 0], px, uw2[:, :, 1], AL.add)
            V.tensor_tensor(pp2[:, :, 1], py, uw2[:, :, 0], AL.subtract)
            V.tensor_scalar(xo3[:, :, 0:2], pp2, 1.0, -1.0, AL.min, AL.max)
            t["uw2"] = uw2

        def emit_B(t):
            # covariance (DVE) + mirror writes (ACT) + output DMA
            off, f = t["off"], t["f"]
            P9, Po9, uw2, q = t["P9"], t["Po9"], t["uw2"], t["q"]
            gh = tmp.tile([P, 2 * f], F32, tag="gh")
            gh2 = gh[:].rearrange("p (f d) -> p f d", d=2)
            V.tensor_tensor(
                gh2, uw2,
                P9[:, :, 8].unsqueeze(-1).broadcast_to([P, f, 2]),
                AL.mult,
            )
            D2 = tmp.tile([P, 2 * f], F32, tag="D2")
            D22 = D2[:].rearrange("p (f d) -> p f d", d=2)
            V.tensor_tensor(D22, P9[:, :, 2:6:3], P9[:, :, 6:8], AL.add)
            D01 = tmp.tile([P, f], F32, tag="D01")
            V.tensor_tensor(D01[:], P9[:, :, 1], P9[:, :, 3], AL.add)
            k01 = tmp.tile([P, 2 * f], F32, tag="k01")
            k2 = k01[:].rearrange("p (f d) -> p f d", d=2)
            V.tensor_tensor(k2, D22, gh2, AL.add)
            m01 = tmp.tile([P, 2 * f], F32, tag="m01")
            mm2 = m01[:].rearrange("p (f d) -> p f d", d=2)
            V.tensor_tensor(mm2, uw2, k2, AL.mult)
            pd = tmp.tile([P, 2 * f], F32, tag="pd")
            pd2 = pd[:].rearrange("p (f d) -> p f d", d=2)
            V.tensor_tensor(pd2, P9[:, :, 0:5:4], mm2, AL.add)
            # diag out = (q + 1e-6) + pd
            V.scalar_tensor_tensor(
                Po9[:, :, 0:5:4],
                q[:].unsqueeze(-1).broadcast_to([P, f, 2]),
                1e-6, pd2, AL.add, AL.add,
            )
            # off-diagonals: compute once on DVE, mirror both spots via ACT
            s0212 = tmp.tile([P, 2 * f], F32, tag="s0212")
            sx2 = s0212[:].rearrange("p (f d) -> p f d", d=2)
            V.scalar_tensor_tensor(sx2, D22, 0.5, gh2, AL.mult, AL.add)
            S.activation(
                Po9[:, :, 2:7:4],
                sx2[:, :, 0].unsqueeze(-1).broadcast_to([P, f, 2]),
                AF.Copy, bias=0.0, scale=1.0,
            )
            S.activation(
                Po9[:, :, 5:8:2],
                sx2[:, :, 1].unsqueeze(-1).broadcast_to([P, f, 2]),
                AF.Copy, bias=0.0, scale=1.0,
            )
            uD12 = tmp.tile([P, f], F32, tag="uD12")
            V.tensor_tensor(uD12[:], uw2[:, :, 0], D22[:, :, 1], AL.mult)
            wD02 = tmp.tile([P, f], F32, tag="wD02")
            V.tensor_tensor(wD02[:], uw2[:, :, 1], D22[:, :, 0], AL.mult)
            s1 = tmp.tile([P, f], F32, tag="s1")
            V.tensor_tensor(s1[:], D01[:], uD12[:], AL.add)
            s2 = tmp.tile([P, f], F32, tag="s2")
            V.tensor_tensor(s2[:], s1[:], wD02[:], AL.add)
            uh = tmp.tile([P, f], F32, tag="uh")
            V.tensor_tensor(uh[:], uw2[:, :, 0], gh2[:, :, 1], AL.mult)
            s01 = tmp.tile([P, f], F32, tag="s01")
            V.scalar_tensor_tensor(s01[:], s2[:], 0.5, uh[:], AL.mult, AL.add)
            S.activation(
                Po9[:, :, 1:4:2],
                s01[:].unsqueeze(-1).broadcast_to([P, f, 2]),
                AF.Copy, bias=0.0, scale=1.0,
            )
            # Po (2,2): P22 + (exp(-8) + 1e-6)
            S.activation(Po9[:, :, 8], P9[:, :, 8], AF.Copy, bias=C22, scale=1.0)
            nc.sync.dma_start(dview(xo_d, off, f, 3), t["xo"][:])
            nc.sync.dma_start(dview(Po_d, off, f, 9), t["Po"][:])

        for rep_i in range(reps):
            if nocompute:
                for off, f in tiles:
                    t = emit_load(off, f)
                    nc.vector.tensor_copy(t["xo"][:, 0:1], t["x3"][:, 0:1, 0])
                    nc.vector.tensor_copy(t["Po"][:, 0:1], t["P9"][:, 0:1, 0])
                    nc.sync.dma_start(dview(xo_d, off, f, 3), t["xo"][:])
                    nc.sync.dma_start(dview(Po_d, off, f, 9), t["Po"][:])
                continue
            if pipeline:
                prev = None
                for off, f in tiles:
                    t = emit_load(off, f)
                    emit_A1(t)
                    if prev is not None:
                        emit_B(prev)
                    emit_A2(t)
                    prev = t
                emit_B(prev)
            else:
                for off, f in tiles:
                    t = emit_load(off, f)
                    emit_A1(t)
                    emit_A2(t)
                    emit_B(t)

    nc.compile()
    return nc


def get_nc(bs=BS, fcfg=F, reps=1, nocompute=False, hw_loop=0, bufs=BUFS,
           pipeline=True):
    key = (bs, fcfg if isinstance(fcfg, int) else tuple(fcfg), reps,
           nocompute, hw_loop, bufs, pipeline)
    if key not in _CACHE:
        _CACHE[key] = _build(bs, fcfg, reps, nocompute, hw_loop, bufs, pipeline)
    return _CACHE[key]


def kernel(x, P, a, theta):
    from concourse.bass_utils import run_bass_kernel_spmd

    nc = get_nc()
    in_maps = []
    for c in range(N_CORES):
        s = slice(c * BS, (c + 1) * BS)
        in_maps.append(
            {
                "x": np.ascontiguousarray(x[s]).reshape(-1),
                "P": np.ascontiguousarray(P[s]).reshape(-1),
                "a": np.ascontiguousarray(a[s]).reshape(-1),
                "theta": np.ascontiguousarray(theta[s]).reshape(-1),
            }
        )
    res = run_bass_kernel_spmd(nc, in_maps, core_ids=list(range(N_CORES)))
    x_out = np.concatenate(
        [r["x_out"].reshape(BS, 3) for r in res.results], axis=0
    )
    P_out = np.concatenate(
        [r["P_out"].reshape(BS, 3, 3) for r in res.results], axis=0
    )
    return x_out, P_out
